# revision 5
# baseline (speedup 1.0000x reference)
"""Multi-head causal attention (B=4, S=2048, D=1024, H=16) on 8 trn2 cores.

v3: like v2 (bf16 + FWL, resident weights, SBUF ctx, big exp slabs,
software-pipelined emission) plus:
  - Head-PAIR attention with row-tiled K=64 score matmuls: heads 2t/2t+1
    live in partition halves of the packed kt/qs tiles; their score matmuls
    target PE row-tiles (0,0)/(64,0) and run CONCURRENTLY (2x score
    throughput; zero-padding no longer needed).
  - One [128,1024] PSUM slab per (pair, key-block): panel 0 = even head,
    panel 1 = odd head; one exp covers both heads.
  - Causal trimming on diagonal key-blocks: scores/exp/mask/PV only touch
    queries q >= 128*j (the rest is masked out anyway).
  - PSUM: sp 2x2 banks, cp bufs=3 (two live per pair + transition), pp 1.
"""

import sys

import numpy as np

_BASS_PATH = "/opt/trn_rl_repo"
if _BASS_PATH not in sys.path:
    sys.path.insert(0, _BASS_PATH)

B, S, D, H, DK = 4, 2048, 1024, 16, 64
NCORES = 8
FH = 512
HL = 8
NSC = 4
SQ = 512
NKB = 16
NDM = 8

_cache = {}


def _build():
    import concourse.bacc as bacc
    import concourse.mybir as mybir
    from concourse.tile import TileContext

    f32, bf16 = mybir.dt.float32, mybir.dt.bfloat16
    AF = mybir.ActivationFunctionType

    nc = bacc.Bacc("TRN2", target_bir_lowering=False, debug=False, num_devices=1)

    xq_d = nc.dram_tensor("xq", [D, S], bf16, kind="ExternalInput").ap()
    xk_d = nc.dram_tensor("xk", [D, S], bf16, kind="ExternalInput").ap()
    xv_d = nc.dram_tensor("xv", [D, S], bf16, kind="ExternalInput").ap()
    wq_d = nc.dram_tensor("wq", [D, FH], bf16, kind="ExternalInput").ap()
    wk_d = nc.dram_tensor("wk", [D, FH], bf16, kind="ExternalInput").ap()
    wv_d = nc.dram_tensor("wv", [D, FH], bf16, kind="ExternalInput").ap()
    wo_d = nc.dram_tensor("wo", [FH, D], bf16, kind="ExternalInput").ap()
    # per-j causal masks [128, 4*512]: masks[k, 512*j + q] = (k <= q - 128*j)
    mask_d = nc.dram_tensor("masks", [128, 2048], bf16, kind="ExternalInput").ap()
    bq_d = nc.dram_tensor("bq", [FH], f32, kind="ExternalInput").ap()
    bk_d = nc.dram_tensor("bk", [FH], f32, kind="ExternalInput").ap()
    out_d = nc.dram_tensor("out", [S, D], f32, kind="ExternalOutput").ap()

    with TileContext(nc) as tc:
        with (
            tc.tile_pool(name="res", bufs=1) as res,
            tc.tile_pool(name="st", bufs=1) as st,
            tc.tile_pool(name="psum", bufs=1, space="PSUM") as psp,
        ):
            # ---- resident tiles ----
            # packed K^T per head pair: rows 0-63 = head 2t, 64-127 = head 2t+1
            kt = [
                res.tile([128, S], bf16, name=f"kt{t}", tag=f"kt{t}") for t in range(4)
            ]
            vaug = [
                res.tile([128, 584], bf16, name=f"va{k}", tag=f"va{k}")
                for k in range(NKB)
            ]
            for k in range(NKB):
                nc.vector.memset(vaug[k][:, HL * 65 : 584], 0.0)
            bq_t = [res.tile([128, 1], f32, name=f"bq{i}", tag=f"bq{i}") for i in range(4)]
            bk_t = [res.tile([128, 1], f32, name=f"bk{i}", tag=f"bk{i}") for i in range(4)]
            for i in range(4):
                nc.sync.dma_start(
                    bq_t[i][:],
                    bq_d[i * 128 : (i + 1) * 128].rearrange("(p o) -> p o", o=1),
                )
                nc.sync.dma_start(
                    bk_t[i][:],
                    bk_d[i * 128 : (i + 1) * 128].rearrange("(p o) -> p o", o=1),
                )
            ones_t = res.tile([128, HL], f32, name="ones", tag="ones")
            nc.vector.memset(ones_t[:], 1.0)
            # weight DMAs in first-use order (k, v, q projections run in that
            # order at the start) so the first matmul chain isn't queued
            # behind megabytes of later-needed weights
            wq_sb, wk_sb, wv_sb = [], [], []
            for pname, w_d, lst in (
                ("k", wk_d, wk_sb),
                ("v", wv_d, wv_sb),
                ("q", wq_d, wq_sb),
            ):
                for dm in range(NDM):
                    wt = res.tile(
                        [128, FH], bf16, name=f"w{pname}{dm}", tag=f"w{pname}{dm}"
                    )
                    nc.sync.dma_start(wt[:], w_d[dm * 128 : (dm + 1) * 128, :])
                    lst.append(wt)
            mask_sb = res.tile([128, 2048], bf16, name="mask_sb", tag="mask_sb")
            nc.sync.dma_start(mask_sb[:], mask_d[:])
            wo_sb = []
            for fc in range(4):
                wt = res.tile([128, D], bf16, name=f"wo{fc}", tag=f"wo{fc}")
                nc.sync.dma_start(wt[:], wo_d[fc * 128 : (fc + 1) * 128, :])
                wo_sb.append(wt)
            qs = [
                [
                    res.tile([128, SQ], bf16, name=f"qs{sc}_{i}", tag=f"qs{sc}_{i}")
                    for i in range(4)
                ]
                for sc in range(NSC)
            ]
            ctx = [
                res.tile([128, S], bf16, name=f"ctx{fc}", tag=f"ctx{fc}")
                for fc in range(4)
            ]

            # ---- filler units ----
            def make_proj_units(sc):
                units = []
                for pname, x_d, w_sb in (
                    ("k", xk_d, wk_sb),
                    ("v", xv_d, wv_sb),
                    ("q", xq_d, wq_sb),
                ):
                    box = {}

                    def load(pname=pname, x_d=x_d, box=box, sc=sc):
                        xr = []
                        for dm in range(NDM):
                            xt = st.tile(
                                [128, SQ], bf16, name=f"x{dm}", tag=f"x{dm}", bufs=2
                            )
                            nc.sync.dma_start(
                                xt[:],
                                x_d[dm * 128 : (dm + 1) * 128, sc * SQ : (sc + 1) * SQ],
                            )
                            xr.append(xt)
                        box["x"] = xr

                    for gi in range(4):
                        for half in range(2):

                            def unit(
                                pname=pname,
                                w_sb=w_sb,
                                gi=gi,
                                half=half,
                                box=box,
                                sc=sc,
                                load=load,
                            ):
                                if gi == 0 and half == 0:
                                    load()
                                xr = box["x"]
                                if half == 0:
                                    box["pp"] = psp.tile(
                                        [128, SQ], f32, name="pp", tag="pp", bufs=1
                                    )
                                pp = box["pp"]
                                dms = range(4 * half, 4 * half + 4)
                                if pname in ("q", "k"):
                                    for dm in dms:
                                        nc.tensor.matmul(
                                            pp[:],
                                            w_sb[dm][:, gi * 128 : (gi + 1) * 128],
                                            xr[dm][:],
                                            start=(dm == 0),
                                            stop=(dm == NDM - 1),
                                        )
                                else:
                                    for dm in dms:
                                        nc.tensor.matmul(
                                            pp[:],
                                            xr[dm][:, gi * 128 : (gi + 1) * 128],
                                            w_sb[dm][:],
                                            start=(dm == 0),
                                            stop=(dm == NDM - 1),
                                        )
                                if half == 0:
                                    return
                                if pname == "k":
                                    nc.scalar.activation(
                                        kt[gi][:, sc * SQ : (sc + 1) * SQ],
                                        pp[:],
                                        AF.Identity,
                                        bias=bk_t[gi][:],
                                    )
                                elif pname == "q":
                                    nc.scalar.activation(
                                        qs[sc][gi][:],
                                        pp[:],
                                        AF.Identity,
                                        bias=bq_t[gi][:],
                                    )
                                else:
                                    kb = sc * 4 + gi
                                    va3 = vaug[kb][:, 0 : HL * 65].rearrange(
                                        "p (h e) -> p h e", e=65
                                    )
                                    pp3 = pp[:].rearrange("p (h e) -> p h e", e=64)
                                    nc.vector.tensor_copy(va3[:, :, 0:64], pp3[:])
                                    nc.vector.tensor_copy(
                                        va3[:, :, 64:65],
                                        ones_t[:].rearrange("p (h o) -> p h o", o=1),
                                    )

                            units.append(unit)
                return units

            def make_o_units(sb):
                units = []
                for qb in range(4):
                    for n2 in range(2):

                        def unit(qb=qb, n2=n2, sb=sb):
                            pp = psp.tile([128, SQ], f32, name="pp", tag="pp", bufs=1)
                            for fc in range(4):
                                nc.tensor.matmul(
                                    pp[:],
                                    ctx[fc][:, sb * SQ + qb * 128 : sb * SQ + (qb + 1) * 128],
                                    wo_sb[fc][:, n2 * SQ : (n2 + 1) * SQ],
                                    start=(fc == 0),
                                    stop=(fc == 3),
                                )
                            ob = st.tile([128, SQ], f32, name="ob", tag="ob", bufs=2)
                            nc.vector.tensor_copy(ob[:], pp[:])
                            nc.sync.dma_start(
                                out_d[
                                    sb * SQ + qb * 128 : sb * SQ + (qb + 1) * 128,
                                    n2 * SQ : (n2 + 1) * SQ,
                                ],
                                ob[:],
                            )

                        units.append(unit)
                return units

            # ---- attention: head pair hp (heads 2hp, 2hp+1) ----
            def attn_emit(hp, sb, filler):
                nkb = 4 * (sb + 1)
                # non-diagonal key blocks first, diagonal (masked) last
                kbs = list(range(0, 4 * sb)) + list(range(4 * sb, 4 * sb + 4))
                nsl = nkb
                cps = [
                    psp.tile([128, SQ], f32, name=f"cp{e}", tag="cp", bufs=3)
                    for e in range(2)
                ]
                es_tiles = [None] * nsl
                q0 = [0] * nsl  # valid-query start (causal trim)
                for i, kb in enumerate(kbs):
                    if kb >= 4 * sb:
                        q0[i] = (kb - 4 * sb) * 128

                def emit_scores(i):
                    kb = kbs[i]
                    o = q0[i]
                    n = SQ - o
                    sp = psp.tile([128, 2 * SQ], f32, name="sp", tag="sp", bufs=2)
                    for e in range(2):
                        po = e * 64
                        nc.tensor.matmul(
                            sp[:, e * SQ + o : (e + 1) * SQ],
                            kt[hp][po : po + 64, kb * 128 : (kb + 1) * 128],
                            qs[sb][hp][po : po + 64, o:SQ],
                            start=True,
                            stop=True,
                        )
                    es = st.tile([128, 2 * SQ], bf16, name="es", tag="es", bufs=4)
                    sp3 = sp[:].rearrange("p (o q) -> p o q", o=2)
                    es3 = es[:].rearrange("p (o q) -> p o q", o=2)
                    nc.scalar.activation(es3[:, :, o:SQ], sp3[:, :, o:SQ], AF.Exp)
                    if o or kb >= 4 * sb:
                        j = kb - 4 * sb
                        es2 = st.tile(
                            [128, 2 * SQ], bf16, name="es2", tag="es2", bufs=2
                        )
                        es23 = es2[:].rearrange("p (o q) -> p o q", o=2)
                        m = (
                            mask_sb[:, j * SQ + o : (j + 1) * SQ]
                            .rearrange("p (o q) -> p o q", o=1)
                            .broadcast_to([128, 2, SQ - o])
                        )
                        nc.vector.tensor_mul(es23[:, :, o:SQ], es3[:, :, o:SQ], m)
                        es = es2
                    es_tiles[i] = es

                def emit_pv(i):
                    kb = kbs[i]
                    o = q0[i]
                    es = es_tiles[i]
                    for e in range(2):
                        h = 2 * hp + e
                        nc.tensor.matmul(
                            cps[e][:, o:SQ],
                            vaug[kb][:, h * 65 : h * 65 + 128],
                            es[:, e * SQ + o : (e + 1) * SQ],
                            start=(i == 0),
                            stop=(i == nsl - 1),
                        )

                emit_scores(0)
                for i in range(nsl):
                    if i + 1 < nsl:
                        emit_scores(i + 1)
                    filler()
                    emit_pv(i)
                for e in range(2):
                    h = 2 * hp + e
                    d1 = st.tile([1, SQ], f32, name="d1", tag="d1", bufs=2)
                    nc.vector.tensor_copy(d1[:], cps[e][64:65, :])
                    rc1 = st.tile([1, SQ], f32, name="rc1", tag="rc1", bufs=2)
                    nc.vector.reciprocal_approx_fast(rc1[:], d1[:])
                    rb = st.tile([64, SQ], f32, name="rb", tag="rb", bufs=2)
                    nc.gpsimd.partition_broadcast(rb[:], rc1[:])
                    nc.vector.tensor_mul(
                        ctx[hp][e * 64 : e * 64 + 64, sb * SQ : (sb + 1) * SQ],
                        cps[e][0:64, :],
                        rb[:],
                    )

            # ---- emission schedule ----
            for u in make_proj_units(0):
                u()
            for sb in range(NSC):
                units = []
                if sb < NSC - 1:
                    units += make_proj_units(sb + 1)
                if sb >= 1:
                    units += make_o_units(sb - 1)
                nslabs = 4 * 4 * (sb + 1)
                state = {"ui": 0, "si": 0}

                def filler(units=units, state=state, nslabs=nslabs):
                    state["si"] += 1
                    nu = len(units)
                    while (
                        state["ui"] < nu
                        and (state["ui"] + 1) * nslabs <= state["si"] * nu
                    ):
                        units[state["ui"]]()
                        state["ui"] += 1

                for hp in range(4):
                    attn_emit(hp, sb, filler)
                while state["ui"] < len(units):
                    units[state["ui"]]()
                    state["ui"] += 1
            for u in make_o_units(NSC - 1):
                u()

    nc.compile()
    return nc


def kernel(
    q,
    k,
    v,
    mask=None,
    Wq=None,
    bq=None,
    Wk=None,
    bk=None,
    Wv=None,
    bv=None,
    Wo=None,
    bo=None,
    **_unused,
):
    import ml_dtypes
    from concourse.bass_utils import run_bass_kernel_spmd

    if "nc" not in _cache:
        _cache["nc"] = _build()
    nc = _cache["nc"]

    bf16 = ml_dtypes.bfloat16
    q = np.asarray(q, np.float32)
    k = np.asarray(k, np.float32)
    v = np.asarray(v, np.float32)
    Wq = np.asarray(Wq, np.float32)
    Wk = np.asarray(Wk, np.float32)
    Wv = np.asarray(Wv, np.float32)
    Wo = np.asarray(Wo, np.float32)
    bq = np.zeros(D, np.float32) if bq is None else np.asarray(bq, np.float32)
    bk = np.zeros(D, np.float32) if bk is None else np.asarray(bk, np.float32)
    bv = np.zeros(D, np.float32) if bv is None else np.asarray(bv, np.float32)
    bo = np.zeros(D, np.float32) if bo is None else np.asarray(bo, np.float32)

    # per-j masks: m[k, 512*j + q] = 1 iff k <= q - 128*j
    kk = np.arange(128)[:, None]
    qq = np.arange(512)[None, :]
    masks = np.empty((128, 2048), np.float32)
    for j in range(4):
        masks[:, 512 * j : 512 * (j + 1)] = (kk <= qq - 128 * j).astype(np.float32)
    masks = masks.astype(bf16)

    xT = {}
    for b in range(B):
        xT[("q", b)] = np.ascontiguousarray(q[b].T).astype(bf16)
        xT[("k", b)] = np.ascontiguousarray(k[b].T).astype(bf16)
        xT[("v", b)] = np.ascontiguousarray(v[b].T).astype(bf16)
    wqs, wks, wvs, wos, bqs, bks = {}, {}, {}, {}, {}, {}
    for hg in range(2):
        sl = slice(hg * FH, (hg + 1) * FH)
        wqs[hg] = (np.ascontiguousarray(Wq[sl, :].T) * np.float32(0.125)).astype(bf16)
        wks[hg] = np.ascontiguousarray(Wk[sl, :].T).astype(bf16)
        wvs[hg] = np.ascontiguousarray(Wv[sl, :].T).astype(bf16)
        wos[hg] = np.ascontiguousarray(Wo[:, sl].T).astype(bf16)
        bqs[hg] = np.ascontiguousarray(bq[sl]) * np.float32(0.125)
        bks[hg] = np.ascontiguousarray(bk[sl])

    in_maps = []
    for c in range(NCORES):
        b, hg = c // 2, c % 2
        in_maps.append(
            {
                "xq": xT[("q", b)],
                "xk": xT[("k", b)],
                "xv": xT[("v", b)],
                "wq": wqs[hg],
                "wk": wks[hg],
                "wv": wvs[hg],
                "wo": wos[hg],
                "masks": masks,
                "bq": bqs[hg],
                "bk": bks[hg],
            }
        )

    res = run_bass_kernel_spmd(nc, in_maps, list(range(NCORES)))
    out = np.empty((B, S, D), np.float32)
    for b in range(B):
        out[b] = res.results[2 * b]["out"] + res.results[2 * b + 1]["out"]
    const = Wo @ bv + bo
    if np.any(const):
        out += const[None, None, :]
    return out


# revision 18
# speedup vs baseline: 1.1719x; 1.1719x over previous
"""Multi-head causal attention (B=4, S=2048, D=1024, H=16) on 8 trn2 cores.

Sharding: tensor-parallel over heads x data-parallel over batch.
core c -> (batch b = c//2, head-group hg = c%2 of 8 heads). Every core runs
an identical SPMD program on different data. Host sums the two partial
outputs per batch and folds the Wo @ bv + bo constant.

v2 design (vs f32r baseline at ~470us):
  - All matmul data is bf16 (tolerance is 2e-2; bf16 lands ~2e-3). bf16
    enables Fast Weight Load (f32r is fp32-class -> no FWL), which removes
    the ~60ns/MM LDWEIGHTS overhead observed in the baseline trace.
  - Score matmuls are K=128 zero-padded: per-head K^T tiles (ktp) hold the
    64 dk rows in the half of the partition range matching the head's rows
    in the packed Q tile, zeros elsewhere. Zeros contribute exactly 0, and
    every matmul is a full-K=128 N=512 stream => PE activity monitor (HAM)
    sees a dense stream and holds the 2.4 GHz clock.
  - Weights resident in SBUF (loaded once); ctx kept in SBUF (no DRAM
    spill/reload of the per-head context).
  - Attention processed in 2-key-block slabs: scores into a [128,1024]
    2-bank PSUM tile, ONE exp per slab (FD=1024 amortizes ScalarE's
    ~352-cycle per-instruction overhead), masked diag slabs via one bf16
    DVE multiply against a precomputed [128,2048] slab mask.
  - Softmax denominators via the ones-column trick in the PV stationary
    (V augmented to 65 cols/head); normalization = DVE reciprocal of the
    PSUM denominator row -> gpsimd partition-broadcast -> DVE multiply,
    written straight into the SBUF ctx tiles.
  - Emission is software-pipelined: scores(i+1) and a projection/output
    filler chunk are emitted between scores(i) and PV(i), so the PE never
    waits on the exp; QKV projections for the next superblock and the
    output projection for the previous one ride along as filler.
"""

import sys

import numpy as np

_BASS_PATH = "/opt/trn_rl_repo"
if _BASS_PATH not in sys.path:
    sys.path.insert(0, _BASS_PATH)

B, S, D, H, DK = 4, 2048, 1024, 16, 64
NCORES = 8
FH = 512  # features per core (8 heads)
HL = 8  # local heads
NSC = 4  # seq superblocks of 512
SQ = 512
NKB = 16  # key blocks of 128
NDM = 8  # d_model chunks of 128

_cache = {}


def _build():
    import concourse.bacc as bacc
    import concourse.mybir as mybir
    from concourse.tile import TileContext

    f32, bf16 = mybir.dt.float32, mybir.dt.bfloat16
    AF = mybir.ActivationFunctionType

    nc = bacc.Bacc("TRN2", target_bir_lowering=False, debug=False, num_devices=1)

    xq_d = nc.dram_tensor("xq", [D, S], bf16, kind="ExternalInput").ap()
    xk_d = nc.dram_tensor("xk", [D, S], bf16, kind="ExternalInput").ap()
    xv_d = nc.dram_tensor("xv", [D, S], bf16, kind="ExternalInput").ap()
    wq_d = nc.dram_tensor("wq", [D, FH], bf16, kind="ExternalInput").ap()
    wk_d = nc.dram_tensor("wk", [D, FH], bf16, kind="ExternalInput").ap()
    wv_d = nc.dram_tensor("wv", [D, FH], bf16, kind="ExternalInput").ap()
    wo_d = nc.dram_tensor("wo", [FH, D], bf16, kind="ExternalInput").ap()
    # slab masks [128, 2048]: cols 1024*d + 512*p + q hold the 0/1 causal
    # mask for diagonal key-block j = 2d+p: m = (k <= q - 128*j)
    mask_d = nc.dram_tensor("masks", [128, 2048], bf16, kind="ExternalInput").ap()
    bq_d = nc.dram_tensor("bq", [FH], f32, kind="ExternalInput").ap()
    bk_d = nc.dram_tensor("bk", [FH], f32, kind="ExternalInput").ap()
    out_d = nc.dram_tensor("out", [S, D], bf16, kind="ExternalOutput").ap()

    with TileContext(nc) as tc:
        with (
            tc.tile_pool(name="res", bufs=1) as res,
            tc.tile_pool(name="st", bufs=1) as st,
            tc.tile_pool(name="psum", bufs=1, space="PSUM") as psp,
        ):
            # ---- resident tiles ----
            # per-head K^T, zero-padded to K=128: even heads use partitions
            # 0-63 (matching their rows in the packed Q tile), odd heads
            # 64-127; the other half stays zero.
            ktp = [
                res.tile([128, S], bf16, name=f"ktp{h}", tag=f"ktp{h}")
                for h in range(HL)
            ]
            for h in range(HL):
                z = slice(64, 128) if h % 2 == 0 else slice(0, 64)
                nc.vector.memset(ktp[h][z, :], 0.0)
            # V augmented: 8 heads x (64 V cols + ones col) + pad
            vaug = [
                res.tile([128, 584], bf16, name=f"va{k}", tag=f"va{k}")
                for k in range(NKB)
            ]
            for k in range(NKB):
                # pad cols (past the 8*65 data cols) are read by head 7's
                # 128-wide PV stationary slice; zero them once
                nc.vector.memset(vaug[k][:, HL * 65 : 584], 0.0)
            bq_t = [res.tile([128, 1], f32, name=f"bq{i}", tag=f"bq{i}") for i in range(4)]
            bk_t = [res.tile([128, 1], f32, name=f"bk{i}", tag=f"bk{i}") for i in range(4)]
            for i in range(4):
                nc.sync.dma_start(
                    bq_t[i][:],
                    bq_d[i * 128 : (i + 1) * 128].rearrange("(p o) -> p o", o=1),
                )
                nc.sync.dma_start(
                    bk_t[i][:],
                    bk_d[i * 128 : (i + 1) * 128].rearrange("(p o) -> p o", o=1),
                )
            ones_t = res.tile([128, HL], f32, name="ones", tag="ones")
            nc.vector.memset(ones_t[:], 1.0)
            # resident weights: tiles created here, DMAs issued lazily in
            # each projection's first unit (interleaved with its x loads) so
            # the first matmul chain isn't queued behind megabytes of
            # later-needed weights
            wq_sb, wk_sb, wv_sb = [], [], []
            wdram = {}
            for pname, w_d, lst in (
                ("k", wk_d, wk_sb),
                ("v", wv_d, wv_sb),
                ("q", wq_d, wq_sb),
            ):
                wdram[pname] = w_d
                for dm in range(NDM):
                    wt = res.tile(
                        [128, FH], bf16, name=f"w{pname}{dm}", tag=f"w{pname}{dm}"
                    )
                    lst.append(wt)
            mask_sb = res.tile([128, 2048], bf16, name="mask_sb", tag="mask_sb")
            wo_sb = [
                res.tile([128, D], bf16, name=f"wo{fc}", tag=f"wo{fc}")
                for fc in range(4)
            ]

            def load_late_residents():
                nc.sync.dma_start(mask_sb[:], mask_d[:])
                for fc in range(4):
                    nc.sync.dma_start(
                        wo_sb[fc][:], wo_d[fc * 128 : (fc + 1) * 128, :]
                    )
            # per-superblock packed Q (2 heads per tile), resident
            qs = [
                [
                    res.tile([128, SQ], bf16, name=f"qs{sc}_{i}", tag=f"qs{sc}_{i}")
                    for i in range(4)
                ]
                for sc in range(NSC)
            ]
            # ctx^T in SBUF: 4 tiles [128 feats, S]
            ctx = [
                res.tile([128, S], bf16, name=f"ctx{fc}", tag=f"ctx{fc}")
                for fc in range(4)
            ]

            # ---- filler units (projection / output-projection chunks) ----
            def make_proj_units(sc):
                """QKV projections for superblock sc, as ~4-MM units."""
                units = []
                for pname, x_d, w_sb in (
                    ("k", xk_d, wk_sb),
                    ("v", xv_d, wv_sb),
                    ("q", xq_d, wq_sb),
                ):
                    box = {}

                    def load(pname=pname, x_d=x_d, w_sb=w_sb, box=box, sc=sc):
                        xr = []
                        for dm in range(NDM):
                            if sc == 0:
                                nc.sync.dma_start(
                                    w_sb[dm][:],
                                    wdram[pname][dm * 128 : (dm + 1) * 128, :],
                                )
                            xt = st.tile(
                                [128, SQ],
                                bf16,
                                name=f"x{dm}",
                                tag=f"x{dm}",
                                bufs=2,
                            )
                            nc.sync.dma_start(
                                xt[:],
                                x_d[dm * 128 : (dm + 1) * 128, sc * SQ : (sc + 1) * SQ],
                            )
                            xr.append(xt)
                        box["x"] = xr

                    for gi in range(4):
                        for half in range(2):

                            def unit(
                                pname=pname,
                                w_sb=w_sb,
                                gi=gi,
                                half=half,
                                box=box,
                                sc=sc,
                                load=load,
                            ):
                                if gi == 0 and half == 0:
                                    load()
                                xr = box["x"]
                                if half == 0:
                                    if pname == "v":
                                        box["pp"] = psp.tile(
                                            [128, FH], f32, name="pp", tag="pp", bufs=2
                                        )
                                    else:
                                        box["pp"] = psp.tile(
                                            [128, SQ], f32, name="pp", tag="pp", bufs=2
                                        )
                                pp = box["pp"]
                                dms = range(4 * half, 4 * half + 4)
                                if pname in ("q", "k"):
                                    for dm in dms:
                                        nc.tensor.matmul(
                                            pp[:],
                                            w_sb[dm][:, gi * 128 : (gi + 1) * 128],
                                            xr[dm][:],
                                            start=(dm == 0),
                                            stop=(dm == NDM - 1),
                                        )
                                else:
                                    for dm in dms:
                                        nc.tensor.matmul(
                                            pp[:],
                                            xr[dm][:, gi * 128 : (gi + 1) * 128],
                                            w_sb[dm][:],
                                            start=(dm == 0),
                                            stop=(dm == NDM - 1),
                                        )
                                if half == 0:
                                    return
                                # evict
                                if pname == "k":
                                    # split per head into zero-padded ktp
                                    h0, h1 = 2 * gi, 2 * gi + 1
                                    nc.scalar.activation(
                                        ktp[h0][0:64, sc * SQ : (sc + 1) * SQ],
                                        pp[0:64, :],
                                        AF.Identity,
                                        bias=bk_t[gi][0:64],
                                    )
                                    nc.scalar.activation(
                                        ktp[h1][64:128, sc * SQ : (sc + 1) * SQ],
                                        pp[64:128, :],
                                        AF.Identity,
                                        bias=bk_t[gi][64:128],
                                    )
                                elif pname == "q":
                                    nc.scalar.activation(
                                        qs[sc][gi][:],
                                        pp[:],
                                        AF.Identity,
                                        bias=bq_t[gi][:],
                                    )
                                else:  # v
                                    kb = sc * 4 + gi
                                    va3 = vaug[kb][:, 0 : HL * 65].rearrange(
                                        "p (h e) -> p h e", e=65
                                    )
                                    pp3 = pp[:].rearrange("p (h e) -> p h e", e=64)
                                    nc.vector.tensor_copy(va3[:, :, 0:64], pp3[:])
                                    nc.vector.tensor_copy(
                                        va3[:, :, 64:65],
                                        ones_t[:].rearrange("p (h o) -> p h o", o=1),
                                    )

                            units.append(unit)
                return units

            def make_o_units(sb):
                """Output projection for superblock sb: 8 units of 4 MMs."""
                units = []
                for qb in range(4):
                    for n2 in range(2):

                        def unit(qb=qb, n2=n2, sb=sb):
                            pp = psp.tile([128, SQ], f32, name="pp", tag="pp", bufs=2)
                            for fc in range(4):
                                nc.tensor.matmul(
                                    pp[:],
                                    ctx[fc][:, sb * SQ + qb * 128 : sb * SQ + (qb + 1) * 128],
                                    wo_sb[fc][:, n2 * SQ : (n2 + 1) * SQ],
                                    start=(fc == 0),
                                    stop=(fc == 3),
                                )
                            ob = st.tile([128, SQ], bf16, name="ob", tag="ob", bufs=2)
                            nc.vector.tensor_copy(ob[:], pp[:])
                            nc.sync.dma_start(
                                out_d[
                                    sb * SQ + qb * 128 : sb * SQ + (qb + 1) * 128,
                                    n2 * SQ : (n2 + 1) * SQ,
                                ],
                                ob[:],
                            )

                        units.append(unit)
                return units

            # ---- attention ----
            def attn_emit(h, sb, filler):
                """Emit attention for (head h, superblock sb), pipelined.

                filler: callable that emits ~4 matmuls of independent work
                when invoked (or nothing if exhausted).
                """
                ti = h // 2
                nkb = 4 * (sb + 1)
                # non-diagonal slabs first, diagonal (masked) last
                kbs = list(range(0, 4 * sb)) + list(range(4 * sb, 4 * sb + 4))
                slabs = [(kbs[i], kbs[i + 1]) for i in range(0, nkb, 2)]
                nsl = len(slabs)
                cp = psp.tile([128, SQ], f32, name="cp", tag="cp", bufs=2)

                es_tiles = [None] * nsl

                def emit_scores(i):
                    kb0, kb1 = slabs[i]
                    diag = kb0 >= 4 * sb
                    d = (kb0 - 4 * sb) // 2 if diag else 0
                    ou = 256 * d if diag else 0  # union valid-q start (trim)
                    sp = psp.tile([128, 2 * SQ], f32, name="sp", tag="sp", bufs=2)
                    for p, kb in enumerate((kb0, kb1)):
                        nc.tensor.matmul(
                            sp[:, p * SQ + ou : (p + 1) * SQ],
                            ktp[h][:, kb * 128 : (kb + 1) * 128],
                            qs[sb][ti][:, ou:SQ],
                            start=True,
                            stop=True,
                        )
                    es = st.tile([128, 2 * SQ], bf16, name="es", tag="es", bufs=4)
                    if ou == 0:
                        nc.scalar.activation(es[:], sp[:], AF.Exp)
                    else:
                        sp3 = sp[:].rearrange("p (o q) -> p o q", o=2)
                        es3 = es[:].rearrange("p (o q) -> p o q", o=2)
                        nc.scalar.activation(
                            es3[:, :, ou:SQ], sp3[:, :, ou:SQ], AF.Exp
                        )
                    if diag:
                        es2 = st.tile(
                            [128, 2 * SQ], bf16, name="es2", tag="es2", bufs=2
                        )
                        if ou == 0:
                            nc.vector.tensor_mul(
                                es2[:], es[:], mask_sb[:, d * 1024 : (d + 1) * 1024]
                            )
                        else:
                            es3 = es[:].rearrange("p (o q) -> p o q", o=2)
                            es23 = es2[:].rearrange("p (o q) -> p o q", o=2)
                            m3 = mask_sb[:, d * 1024 : (d + 1) * 1024].rearrange(
                                "p (o q) -> p o q", o=2
                            )
                            nc.vector.tensor_mul(
                                es23[:, :, ou:SQ], es3[:, :, ou:SQ], m3[:, :, ou:SQ]
                            )
                        es = es2
                    es_tiles[i] = es

                def emit_pv(i):
                    kb0, kb1 = slabs[i]
                    diag = kb0 >= 4 * sb
                    d = (kb0 - 4 * sb) // 2 if diag else 0
                    es = es_tiles[i]
                    for p, kb in enumerate((kb0, kb1)):
                        op = 128 * (2 * d + p) if diag else 0
                        nc.tensor.matmul(
                            cp[:, op:SQ],
                            vaug[kb][:, h * 65 : h * 65 + 128],
                            es[:, p * SQ + op : (p + 1) * SQ],
                            start=(i == 0 and p == 0),
                            stop=(i == nsl - 1 and p == 1),
                        )

                emit_scores(0)
                for i in range(nsl):
                    if i + 1 < nsl:
                        emit_scores(i + 1)
                    filler()
                    emit_pv(i)
                # normalization: 1/denominator broadcast over the 64 V rows.
                # Stage the PSUM denominator row to SBUF first: custom-DVE
                # ops (reciprocal_approx_fast) need SBUF operands on HW.
                d1 = st.tile([1, SQ], f32, name="d1", tag="d1", bufs=2)
                nc.vector.tensor_copy(d1[:], cp[64:65, :])
                rc1 = st.tile([1, SQ], f32, name="rc1", tag="rc1", bufs=2)
                nc.vector.reciprocal_approx_fast(rc1[:], d1[:])
                rb = st.tile([64, SQ], f32, name="rb", tag="rb", bufs=2)
                nc.gpsimd.partition_broadcast(rb[:], rc1[:])
                nc.vector.tensor_mul(
                    ctx[ti][(h % 2) * 64 : (h % 2) * 64 + 64, sb * SQ : (sb + 1) * SQ],
                    cp[0:64, :],
                    rb[:],
                )

            # ---- emission schedule ----
            for u in make_proj_units(0):
                u()
            load_late_residents()
            for sb in range(NSC):
                # fillers: next superblock's projections; all deferred output
                # projections go into sb=3 (the only ACT-bound stretch)
                units = []
                if sb < NSC - 1:
                    units += make_proj_units(sb + 1)
                else:
                    for s2 in range(NSC - 1):
                        units += make_o_units(s2)
                nslabs = HL * 2 * (sb + 1)
                state = {"ui": 0, "si": 0}

                def filler(units=units, state=state, nslabs=nslabs):
                    state["si"] += 1
                    nu = len(units)
                    while (
                        state["ui"] < nu
                        and (state["ui"] + 1) * nslabs <= state["si"] * nu
                    ):
                        units[state["ui"]]()
                        state["ui"] += 1

                for h in range(HL):
                    attn_emit(h, sb, filler)
                while state["ui"] < len(units):
                    units[state["ui"]]()
                    state["ui"] += 1
            for u in make_o_units(NSC - 1):
                u()

    nc.compile()
    return nc


def kernel(
    q,
    k,
    v,
    mask=None,
    Wq=None,
    bq=None,
    Wk=None,
    bk=None,
    Wv=None,
    bv=None,
    Wo=None,
    bo=None,
    **_unused,
):
    import ml_dtypes
    from concourse.bass_utils import run_bass_kernel_spmd

    if "nc" not in _cache:
        _cache["nc"] = _build()
    nc = _cache["nc"]

    bf16 = ml_dtypes.bfloat16
    q = np.asarray(q, np.float32)
    k = np.asarray(k, np.float32)
    v = np.asarray(v, np.float32)
    Wq = np.asarray(Wq, np.float32)
    Wk = np.asarray(Wk, np.float32)
    Wv = np.asarray(Wv, np.float32)
    Wo = np.asarray(Wo, np.float32)
    bq = np.zeros(D, np.float32) if bq is None else np.asarray(bq, np.float32)
    bk = np.zeros(D, np.float32) if bk is None else np.asarray(bk, np.float32)
    bv = np.zeros(D, np.float32) if bv is None else np.asarray(bv, np.float32)
    bo = np.zeros(D, np.float32) if bo is None else np.asarray(bo, np.float32)

    # slab masks: m2[k, 1024*d + 512*p + q] = 1 iff k <= q - 128*(2d+p)
    kk = np.arange(128)[:, None]
    masks = np.empty((128, 2048), np.float32)
    qq = np.arange(512)[None, :]
    for d in range(2):
        for p in range(2):
            j = 2 * d + p
            masks[:, 1024 * d + 512 * p : 1024 * d + 512 * p + 512] = (
                kk <= qq - 128 * j
            ).astype(np.float32)
    masks = masks.astype(bf16)

    xT = {}
    for b in range(B):
        xT[("q", b)] = np.ascontiguousarray(q[b].T).astype(bf16)
        xT[("k", b)] = np.ascontiguousarray(k[b].T).astype(bf16)
        xT[("v", b)] = np.ascontiguousarray(v[b].T).astype(bf16)
    wqs, wks, wvs, wos, bqs, bks = {}, {}, {}, {}, {}, {}
    for hg in range(2):
        sl = slice(hg * FH, (hg + 1) * FH)
        wqs[hg] = (np.ascontiguousarray(Wq[sl, :].T) * np.float32(0.125)).astype(bf16)
        wks[hg] = np.ascontiguousarray(Wk[sl, :].T).astype(bf16)
        wvs[hg] = np.ascontiguousarray(Wv[sl, :].T).astype(bf16)
        wos[hg] = np.ascontiguousarray(Wo[:, sl].T).astype(bf16)
        bqs[hg] = np.ascontiguousarray(bq[sl]) * np.float32(0.125)
        bks[hg] = np.ascontiguousarray(bk[sl])

    in_maps = []
    for c in range(NCORES):
        b, hg = c // 2, c % 2
        in_maps.append(
            {
                "xq": xT[("q", b)],
                "xk": xT[("k", b)],
                "xv": xT[("v", b)],
                "wq": wqs[hg],
                "wk": wks[hg],
                "wv": wvs[hg],
                "wo": wos[hg],
                "masks": masks,
                "bq": bqs[hg],
                "bk": bks[hg],
            }
        )

    res = run_bass_kernel_spmd(nc, in_maps, list(range(NCORES)))
    out = np.empty((B, S, D), np.float32)
    for b in range(B):
        out[b] = res.results[2 * b]["out"].astype(np.float32) + res.results[
            2 * b + 1
        ]["out"].astype(np.float32)
    const = Wo @ bv + bo  # bv/bo contribution (folds exactly through softmax)
    if np.any(const):
        out += const[None, None, :]
    return out


# revision 20
# speedup vs baseline: 1.1763x; 1.0037x over previous
"""Multi-head causal attention (B=4, S=2048, D=1024, H=16) on 8 trn2 cores.

Sharding: tensor-parallel over heads x data-parallel over batch.
core c -> (batch b = c//2, head-group hg = c%2 of 8 heads). Every core runs
an identical SPMD program on different data. Host sums the two partial
outputs per batch and folds the Wo @ bv + bo constant.

v2 design (vs f32r baseline at ~470us):
  - All matmul data is bf16 (tolerance is 2e-2; bf16 lands ~2e-3). bf16
    enables Fast Weight Load (f32r is fp32-class -> no FWL), which removes
    the ~60ns/MM LDWEIGHTS overhead observed in the baseline trace.
  - Score matmuls are K=128 zero-padded: per-head K^T tiles (ktp) hold the
    64 dk rows in the half of the partition range matching the head's rows
    in the packed Q tile, zeros elsewhere. Zeros contribute exactly 0, and
    every matmul is a full-K=128 N=512 stream => PE activity monitor (HAM)
    sees a dense stream and holds the 2.4 GHz clock.
  - Weights resident in SBUF (loaded once); ctx kept in SBUF (no DRAM
    spill/reload of the per-head context).
  - Attention processed in 2-key-block slabs: scores into a [128,1024]
    2-bank PSUM tile, ONE exp per slab (FD=1024 amortizes ScalarE's
    ~352-cycle per-instruction overhead), masked diag slabs via one bf16
    DVE multiply against a precomputed [128,2048] slab mask.
  - Softmax denominators via the ones-column trick in the PV stationary
    (V augmented to 65 cols/head); normalization = DVE reciprocal of the
    PSUM denominator row -> gpsimd partition-broadcast -> DVE multiply,
    written straight into the SBUF ctx tiles.
  - Emission is software-pipelined: scores(i+1) and a projection/output
    filler chunk are emitted between scores(i) and PV(i), so the PE never
    waits on the exp; QKV projections for the next superblock and the
    output projection for the previous one ride along as filler.
"""

import sys

import numpy as np

_BASS_PATH = "/opt/trn_rl_repo"
if _BASS_PATH not in sys.path:
    sys.path.insert(0, _BASS_PATH)

B, S, D, H, DK = 4, 2048, 1024, 16, 64
NCORES = 8
FH = 512  # features per core (8 heads)
HL = 8  # local heads
NSC = 4  # seq superblocks of 512
SQ = 512
NKB = 16  # key blocks of 128
NDM = 8  # d_model chunks of 128

_cache = {}


def _build():
    import concourse.bacc as bacc
    import concourse.mybir as mybir
    from concourse.tile import TileContext

    f32, bf16 = mybir.dt.float32, mybir.dt.bfloat16
    AF = mybir.ActivationFunctionType

    nc = bacc.Bacc("TRN2", target_bir_lowering=False, debug=False, num_devices=1)

    xq_d = nc.dram_tensor("xq", [D, S], bf16, kind="ExternalInput").ap()
    xk_d = nc.dram_tensor("xk", [D, S], bf16, kind="ExternalInput").ap()
    xv_d = nc.dram_tensor("xv", [D, S], bf16, kind="ExternalInput").ap()
    wq_d = nc.dram_tensor("wq", [D, FH], bf16, kind="ExternalInput").ap()
    wk_d = nc.dram_tensor("wk", [D, FH], bf16, kind="ExternalInput").ap()
    wv_d = nc.dram_tensor("wv", [D, FH], bf16, kind="ExternalInput").ap()
    wo_d = nc.dram_tensor("wo", [FH, D], bf16, kind="ExternalInput").ap()
    # slab masks [128, 2048]: cols 1024*d + 512*p + q hold the 0/1 causal
    # mask for diagonal key-block j = 2d+p: m = (k <= q - 128*j)
    mask_d = nc.dram_tensor("masks", [128, 2048], bf16, kind="ExternalInput").ap()
    bq_d = nc.dram_tensor("bq", [FH], f32, kind="ExternalInput").ap()
    bk_d = nc.dram_tensor("bk", [FH], f32, kind="ExternalInput").ap()
    out_d = nc.dram_tensor("out", [S, D], bf16, kind="ExternalOutput").ap()

    with TileContext(nc) as tc:
        with (
            tc.tile_pool(name="res", bufs=1) as res,
            tc.tile_pool(name="st", bufs=1) as st,
            tc.tile_pool(name="psum", bufs=1, space="PSUM") as psp,
        ):
            # ---- resident tiles ----
            # per-head K^T, zero-padded to K=128: even heads use partitions
            # 0-63 (matching their rows in the packed Q tile), odd heads
            # 64-127; the other half stays zero.
            ktp = [
                res.tile([128, S], bf16, name=f"ktp{h}", tag=f"ktp{h}")
                for h in range(HL)
            ]
            for h in range(HL):
                z = slice(64, 128) if h % 2 == 0 else slice(0, 64)
                nc.vector.memset(ktp[h][z, :], 0.0)
            # V augmented: 8 heads x (64 V cols + ones col) + pad
            vaug = [
                res.tile([128, 584], bf16, name=f"va{k}", tag=f"va{k}")
                for k in range(NKB)
            ]
            for k in range(NKB):
                # pad cols (past the 8*65 data cols) are read by head 7's
                # 128-wide PV stationary slice; zero them once
                nc.vector.memset(vaug[k][:, HL * 65 : 584], 0.0)
            bq_t = [res.tile([128, 1], f32, name=f"bq{i}", tag=f"bq{i}") for i in range(4)]
            bk_t = [res.tile([128, 1], f32, name=f"bk{i}", tag=f"bk{i}") for i in range(4)]
            for i in range(4):
                nc.sync.dma_start(
                    bq_t[i][:],
                    bq_d[i * 128 : (i + 1) * 128].rearrange("(p o) -> p o", o=1),
                )
                nc.sync.dma_start(
                    bk_t[i][:],
                    bk_d[i * 128 : (i + 1) * 128].rearrange("(p o) -> p o", o=1),
                )
            ones_t = res.tile([128, HL], f32, name="ones", tag="ones")
            nc.vector.memset(ones_t[:], 1.0)
            # resident weights: tiles created here, DMAs issued lazily in
            # each projection's first unit (interleaved with its x loads) so
            # the first matmul chain isn't queued behind megabytes of
            # later-needed weights
            wq_sb, wk_sb, wv_sb = [], [], []
            wdram = {}
            for pname, w_d, lst in (
                ("k", wk_d, wk_sb),
                ("v", wv_d, wv_sb),
                ("q", wq_d, wq_sb),
            ):
                wdram[pname] = w_d
                for dm in range(NDM):
                    wt = res.tile(
                        [128, FH], bf16, name=f"w{pname}{dm}", tag=f"w{pname}{dm}"
                    )
                    lst.append(wt)
            mask_sb = res.tile([128, 2048], bf16, name="mask_sb", tag="mask_sb")
            wo_sb = [
                res.tile([128, D], bf16, name=f"wo{fc}", tag=f"wo{fc}")
                for fc in range(4)
            ]

            def load_late_residents():
                nc.sync.dma_start(mask_sb[:], mask_d[:])
                for fc in range(4):
                    nc.sync.dma_start(
                        wo_sb[fc][:], wo_d[fc * 128 : (fc + 1) * 128, :]
                    )
            # per-superblock packed Q (2 heads per tile), resident
            qs = [
                [
                    res.tile([128, SQ], bf16, name=f"qs{sc}_{i}", tag=f"qs{sc}_{i}")
                    for i in range(4)
                ]
                for sc in range(NSC)
            ]
            # ctx^T in SBUF: 4 tiles [128 feats, S]
            ctx = [
                res.tile([128, S], bf16, name=f"ctx{fc}", tag=f"ctx{fc}")
                for fc in range(4)
            ]

            # ---- filler units (projection / output-projection chunks) ----
            def make_proj_units(sc):
                """QKV projections for superblock sc, as ~4-MM units."""
                units = []
                for pname, x_d, w_sb in (
                    ("k", xk_d, wk_sb),
                    ("v", xv_d, wv_sb),
                    ("q", xq_d, wq_sb),
                ):
                    box = {}

                    def load(pname=pname, x_d=x_d, w_sb=w_sb, box=box, sc=sc):
                        xr = []
                        for dm in range(NDM):
                            if sc == 0:
                                nc.sync.dma_start(
                                    w_sb[dm][:],
                                    wdram[pname][dm * 128 : (dm + 1) * 128, :],
                                )
                            xt = st.tile(
                                [128, SQ],
                                bf16,
                                name=f"x{dm}",
                                tag=f"x{dm}",
                                bufs=2,
                            )
                            nc.sync.dma_start(
                                xt[:],
                                x_d[dm * 128 : (dm + 1) * 128, sc * SQ : (sc + 1) * SQ],
                            )
                            xr.append(xt)
                        box["x"] = xr

                    for gi in range(4):
                        for half in range(2):

                            def unit(
                                pname=pname,
                                w_sb=w_sb,
                                gi=gi,
                                half=half,
                                box=box,
                                sc=sc,
                                load=load,
                            ):
                                if gi == 0 and half == 0:
                                    load()
                                xr = box["x"]
                                if half == 0:
                                    if pname == "v":
                                        box["pp"] = psp.tile(
                                            [128, FH], f32, name="pp", tag="pp", bufs=2
                                        )
                                    else:
                                        box["pp"] = psp.tile(
                                            [128, SQ], f32, name="pp", tag="pp", bufs=2
                                        )
                                pp = box["pp"]
                                dms = range(4 * half, 4 * half + 4)
                                if pname in ("q", "k"):
                                    for dm in dms:
                                        nc.tensor.matmul(
                                            pp[:],
                                            w_sb[dm][:, gi * 128 : (gi + 1) * 128],
                                            xr[dm][:],
                                            start=(dm == 0),
                                            stop=(dm == NDM - 1),
                                        )
                                else:
                                    for dm in dms:
                                        nc.tensor.matmul(
                                            pp[:],
                                            xr[dm][:, gi * 128 : (gi + 1) * 128],
                                            w_sb[dm][:],
                                            start=(dm == 0),
                                            stop=(dm == NDM - 1),
                                        )
                                if half == 0:
                                    return
                                # evict
                                if pname == "k":
                                    # split per head into zero-padded ktp
                                    h0, h1 = 2 * gi, 2 * gi + 1
                                    nc.scalar.activation(
                                        ktp[h0][0:64, sc * SQ : (sc + 1) * SQ],
                                        pp[0:64, :],
                                        AF.Identity,
                                        bias=bk_t[gi][0:64],
                                    )
                                    nc.scalar.activation(
                                        ktp[h1][64:128, sc * SQ : (sc + 1) * SQ],
                                        pp[64:128, :],
                                        AF.Identity,
                                        bias=bk_t[gi][64:128],
                                    )
                                elif pname == "q":
                                    nc.scalar.activation(
                                        qs[sc][gi][:],
                                        pp[:],
                                        AF.Identity,
                                        bias=bq_t[gi][:],
                                    )
                                else:  # v
                                    kb = sc * 4 + gi
                                    va3 = vaug[kb][:, 0 : HL * 65].rearrange(
                                        "p (h e) -> p h e", e=65
                                    )
                                    pp3 = pp[:].rearrange("p (h e) -> p h e", e=64)
                                    nc.vector.tensor_copy(va3[:, :, 0:64], pp3[:])
                                    nc.vector.tensor_copy(
                                        va3[:, :, 64:65],
                                        ones_t[:].rearrange("p (h o) -> p h o", o=1),
                                    )

                            units.append(unit)
                return units

            def make_o_units(sb):
                """Output projection for superblock sb: 8 units of 4 MMs."""
                units = []
                for qb in range(4):
                    for n2 in range(2):

                        def unit(qb=qb, n2=n2, sb=sb):
                            pp = psp.tile([128, SQ], f32, name="pp", tag="pp", bufs=2)
                            for fc in range(4):
                                nc.tensor.matmul(
                                    pp[:],
                                    ctx[fc][:, sb * SQ + qb * 128 : sb * SQ + (qb + 1) * 128],
                                    wo_sb[fc][:, n2 * SQ : (n2 + 1) * SQ],
                                    start=(fc == 0),
                                    stop=(fc == 3),
                                )
                            ob = st.tile([128, SQ], bf16, name="ob", tag="ob", bufs=2)
                            nc.vector.tensor_copy(ob[:], pp[:])
                            nc.sync.dma_start(
                                out_d[
                                    sb * SQ + qb * 128 : sb * SQ + (qb + 1) * 128,
                                    n2 * SQ : (n2 + 1) * SQ,
                                ],
                                ob[:],
                            )

                        units.append(unit)
                return units

            # ---- attention ----
            def attn_emit(h, sb, filler):
                """Emit attention for (head h, superblock sb), pipelined.

                filler: callable that emits ~4 matmuls of independent work
                when invoked (or nothing if exhausted).
                """
                ti = h // 2
                nkb = 4 * (sb + 1)
                # non-diagonal slabs first, diagonal (masked) last
                kbs = list(range(0, 4 * sb)) + list(range(4 * sb, 4 * sb + 4))
                slabs = [(kbs[i], kbs[i + 1]) for i in range(0, nkb, 2)]
                nsl = len(slabs)
                cp = psp.tile([128, SQ], f32, name="cp", tag="cp", bufs=2)

                es_tiles = [None] * nsl

                def emit_scores(i):
                    kb0, kb1 = slabs[i]
                    diag = kb0 >= 4 * sb
                    d = (kb0 - 4 * sb) // 2 if diag else 0
                    ou = 256 * d if diag else 0  # union valid-q start (trim)
                    sp = psp.tile([128, 2 * SQ], f32, name="sp", tag="sp", bufs=2)
                    for p, kb in enumerate((kb0, kb1)):
                        nc.tensor.matmul(
                            sp[:, p * SQ + ou : (p + 1) * SQ],
                            ktp[h][:, kb * 128 : (kb + 1) * 128],
                            qs[sb][ti][:, ou:SQ],
                            start=True,
                            stop=True,
                        )
                    es = st.tile([128, 2 * SQ], bf16, name="es", tag="es", bufs=4)
                    if ou == 0:
                        nc.scalar.activation(es[:], sp[:], AF.Exp)
                    else:
                        sp3 = sp[:].rearrange("p (o q) -> p o q", o=2)
                        es3 = es[:].rearrange("p (o q) -> p o q", o=2)
                        nc.scalar.activation(
                            es3[:, :, ou:SQ], sp3[:, :, ou:SQ], AF.Exp
                        )
                    if diag:
                        es2 = st.tile(
                            [128, 2 * SQ], bf16, name="es2", tag="es2", bufs=2
                        )
                        if ou == 0:
                            nc.vector.tensor_mul(
                                es2[:], es[:], mask_sb[:, d * 1024 : (d + 1) * 1024]
                            )
                        else:
                            es3 = es[:].rearrange("p (o q) -> p o q", o=2)
                            es23 = es2[:].rearrange("p (o q) -> p o q", o=2)
                            m3 = mask_sb[:, d * 1024 : (d + 1) * 1024].rearrange(
                                "p (o q) -> p o q", o=2
                            )
                            nc.vector.tensor_mul(
                                es23[:, :, ou:SQ], es3[:, :, ou:SQ], m3[:, :, ou:SQ]
                            )
                        es = es2
                    es_tiles[i] = es

                def emit_pv(i):
                    kb0, kb1 = slabs[i]
                    diag = kb0 >= 4 * sb
                    d = (kb0 - 4 * sb) // 2 if diag else 0
                    es = es_tiles[i]
                    for p, kb in enumerate((kb0, kb1)):
                        op = 128 * (2 * d + p) if diag else 0
                        nc.tensor.matmul(
                            cp[:, op:SQ],
                            vaug[kb][:, h * 65 : h * 65 + 128],
                            es[:, p * SQ + op : (p + 1) * SQ],
                            start=(i == 0 and p == 0),
                            stop=(i == nsl - 1 and p == 1),
                        )

                emit_scores(0)
                for i in range(nsl):
                    if i + 1 < nsl:
                        emit_scores(i + 1)
                    filler()
                    emit_pv(i)
                # normalization: 1/denominator broadcast over the 64 V rows.
                # Stage the PSUM denominator row to SBUF first: custom-DVE
                # ops (reciprocal_approx_fast) need SBUF operands on HW.
                d1 = st.tile([1, SQ], f32, name="d1", tag="d1", bufs=2)
                nc.vector.tensor_copy(d1[:], cp[64:65, :])
                rc1 = st.tile([1, SQ], f32, name="rc1", tag="rc1", bufs=2)
                nc.vector.reciprocal_approx_fast(rc1[:], d1[:])
                rb = st.tile([64, SQ], f32, name="rb", tag="rb", bufs=2)
                nc.gpsimd.partition_broadcast(rb[:], rc1[:])
                nc.vector.tensor_mul(
                    ctx[ti][(h % 2) * 64 : (h % 2) * 64 + 64, sb * SQ : (sb + 1) * SQ],
                    cp[0:64, :],
                    rb[:],
                )

            # ---- emission schedule ----
            for u in make_proj_units(0):
                u()
            load_late_residents()
            for sb in range(NSC):
                # fillers: next superblock's projections; all deferred output
                # projections go into sb=3 (the only ACT-bound stretch)
                units = []
                if sb < NSC - 1:
                    units += make_proj_units(sb + 1)
                else:
                    for s2 in range(NSC - 1):
                        units += make_o_units(s2)
                nslabs = HL * 2 * (sb + 1)
                state = {"ui": 0, "si": 0}

                def filler(units=units, state=state, nslabs=nslabs):
                    state["si"] += 1
                    nu = len(units)
                    while (
                        state["ui"] < nu
                        and (state["ui"] + 1) * nslabs <= state["si"] * nu
                    ):
                        units[state["ui"]]()
                        state["ui"] += 1

                for h in range(HL):
                    attn_emit(h, sb, filler)
                while state["ui"] < len(units):
                    units[state["ui"]]()
                    state["ui"] += 1
            for u in make_o_units(NSC - 1):
                u()

    nc.compile()
    return nc


def kernel(
    q,
    k,
    v,
    mask=None,
    Wq=None,
    bq=None,
    Wk=None,
    bk=None,
    Wv=None,
    bv=None,
    Wo=None,
    bo=None,
    **_unused,
):
    import ml_dtypes
    from concourse.bass_utils import run_bass_kernel_spmd

    if "nc" not in _cache:
        _cache["nc"] = _build()
    nc = _cache["nc"]

    bf16 = ml_dtypes.bfloat16
    q = np.asarray(q, np.float32)
    k = np.asarray(k, np.float32)
    v = np.asarray(v, np.float32)
    Wq = np.asarray(Wq, np.float32)
    Wk = np.asarray(Wk, np.float32)
    Wv = np.asarray(Wv, np.float32)
    Wo = np.asarray(Wo, np.float32)
    bq = np.zeros(D, np.float32) if bq is None else np.asarray(bq, np.float32)
    bk = np.zeros(D, np.float32) if bk is None else np.asarray(bk, np.float32)
    bv = np.zeros(D, np.float32) if bv is None else np.asarray(bv, np.float32)
    bo = np.zeros(D, np.float32) if bo is None else np.asarray(bo, np.float32)

    # slab masks: m2[k, 1024*d + 512*p + q] = 1 iff k <= q - 128*(2d+p)
    kk = np.arange(128)[:, None]
    masks = np.empty((128, 2048), np.float32)
    qq = np.arange(512)[None, :]
    for d in range(2):
        for p in range(2):
            j = 2 * d + p
            masks[:, 1024 * d + 512 * p : 1024 * d + 512 * p + 512] = (
                kk <= qq - 128 * j
            ).astype(np.float32)
    masks = masks.astype(bf16)

    xT = {}
    for b in range(B):
        xT[("q", b)] = np.ascontiguousarray(q[b].T).astype(bf16)
        xT[("k", b)] = np.ascontiguousarray(k[b].T).astype(bf16)
        xT[("v", b)] = np.ascontiguousarray(v[b].T).astype(bf16)
    wqs, wks, wvs, wos, bqs, bks = {}, {}, {}, {}, {}, {}
    for hg in range(2):
        sl = slice(hg * FH, (hg + 1) * FH)
        wqs[hg] = (np.ascontiguousarray(Wq[sl, :].T) * np.float32(0.125)).astype(bf16)
        wks[hg] = np.ascontiguousarray(Wk[sl, :].T).astype(bf16)
        wvs[hg] = np.ascontiguousarray(Wv[sl, :].T).astype(bf16)
        wos[hg] = np.ascontiguousarray(Wo[:, sl].T).astype(bf16)
        bqs[hg] = np.ascontiguousarray(bq[sl]) * np.float32(0.125)
        bks[hg] = np.ascontiguousarray(bk[sl])

    in_maps = []
    for c in range(NCORES):
        b, hg = c // 2, c % 2
        in_maps.append(
            {
                "xq": xT[("q", b)],
                "xk": xT[("k", b)],
                "xv": xT[("v", b)],
                "wq": wqs[hg],
                "wk": wks[hg],
                "wv": wvs[hg],
                "wo": wos[hg],
                "masks": masks,
                "bq": bqs[hg],
                "bk": bks[hg],
            }
        )

    res = run_bass_kernel_spmd(nc, in_maps, list(range(NCORES)))
    out = np.empty((B, S, D), np.float32)
    for b in range(B):
        out[b] = res.results[2 * b]["out"].astype(np.float32) + res.results[
            2 * b + 1
        ]["out"].astype(np.float32)
    const = Wo @ bv + bo  # bv/bo contribution (folds exactly through softmax)
    if np.any(const):
        out += const[None, None, :]
    return out
